# revision 15
# baseline (speedup 1.0000x reference)
"""GATConv block (GAT attention + BatchNorm + leaky_relu) on 8 Trainium2 NeuronCores.

Strategy (graph/data parallel, per sharding hint):
- Nodes are sharded across 8 cores by destination range (12500 nodes each).
- Phase 1 (replicated): each core computes the full feature table
  G2[n] = [xt[n] (128) | a_src[n] (4) | a_dst[n] (4)] via PE matmuls from a
  host-transposed copy of x.
- Phase 2: edges are grouped by destination block (<=128 dst nodes), padded to
  128-edge tiles.  Per tile: indirect-DMA gather of source rows, attention
  score -> exp weight on ACT/DVE, one-hot selection matrix S built on DVE from
  dst offsets, and a PE matmul  out[dst_block] += S.T @ [w*xt | w]  that
  accumulates both numerator and softmax denominator in PSUM.
- Softmax uses exp without max-subtraction (scores are bounded ~[-5.3, 5.3],
  exact same math as the reference up to fp rounding).
- Phase 3: BatchNorm batch stats via ones-vector matmuls accumulated in PSUM,
  AllReduce across the 8 cores, then per-channel affine + leaky_relu.

The Bass program is identical on all 8 cores (SPMD); all data-dependent
structure (edge->tile assignment, per-position tile counts) is host data, with
per-position tile counts equalized across cores by sorting blocks per core by
tile count (position-wise max padding).
"""

import sys

sys.path.insert(0, "/opt/trn_rl_repo")

import numpy as np
from contextlib import ExitStack

import concourse.bass as bass
import concourse.mybir as mybir
import concourse.tile as tile
from concourse import bacc

FP32 = mybir.dt.float32
I32 = mybir.dt.int32

N = 100000
E = 1600000
F_IN = 128
H = 4
C = 32
F_OUT = H * C
NEG = 0.2
EPS = 1e-5
NCORES = 8
GROW = 136  # [xt 128 | a_src 4 | a_dst 4]
P = 128


class Cfg:
    """All host-known compile-time structure for one SPMD program."""

    def __init__(self, n_nodes, npos, t_list, shard, ncores):
        self.n = n_nodes          # global node count (G2 rows)
        self.npos = npos          # positions (dst blocks) per core
        self.t_list = t_list      # tiles per position (same across cores)
        self.shard = shard        # valid nodes per core
        self.ncores = ncores
        self.TT = int(sum(t_list))
        self.offs = np.concatenate([[0], np.cumsum(t_list)]).astype(int)
        self.nxt = (n_nodes + P - 1) // P  # node tiles for phase 1


def preprocess(edge_index, n=N, ncores=NCORES):
    src = np.asarray(edge_index[0]).astype(np.int64)
    dst = np.asarray(edge_index[1]).astype(np.int64)
    order = np.argsort(dst, kind="stable")
    s_src = src[order].astype(np.int32)
    s_dst = dst[order].astype(np.int32)

    shard = n // ncores
    npos = (shard + P - 1) // P

    # blocks: per core, per block: (node_lo, n_nodes, e_lo, e_hi)
    blocks = []
    for c_ in range(ncores):
        lo_n = c_ * shard
        hi_n = lo_n + shard
        bl = []
        for b in range(npos):
            nb_lo = lo_n + b * P
            nb_hi = min(nb_lo + P, hi_n)
            e_lo = int(np.searchsorted(s_dst, nb_lo, "left"))
            e_hi = int(np.searchsorted(s_dst, nb_hi, "left"))
            bl.append((nb_lo, nb_hi - nb_lo, e_lo, e_hi))
        blocks.append(bl)

    tiles = np.zeros((ncores, npos), np.int64)
    for c_ in range(ncores):
        for b in range(npos):
            _, _, e_lo, e_hi = blocks[c_][b]
            tiles[c_, b] = max(1, -(-(e_hi - e_lo) // P))

    # position-wise balancing: sort each core's blocks by tile count desc
    perm = np.argsort(-tiles, axis=1, kind="stable")  # perm[c][g] = block idx
    tiles_sorted = np.take_along_axis(tiles, perm, axis=1)
    t_list = tiles_sorted.max(axis=0)  # [npos]

    cfg = Cfg(n, npos, t_list, shard, ncores)

    # meta per position: [src t | rel t | blknode 1] -> (2t+1) cols
    meta = np.zeros((ncores, P, 2 * cfg.TT + npos), np.int32)
    moffs = np.concatenate([[0], np.cumsum(2 * t_list + 1)]).astype(int)
    cfg.moffs = moffs
    out_map = []  # per core: list of (node_lo, n_valid) per position
    for c_ in range(ncores):
        omap = []
        for g in range(npos):
            b = int(perm[c_, g])
            nb_lo, n_nodes, e_lo, e_hi = blocks[c_][b]
            t = int(t_list[g])
            L = e_hi - e_lo
            # sort edges within the block by src for HBM gather locality
            o2 = np.argsort(s_src[e_lo:e_hi], kind="stable")
            src_p = np.zeros(t * P, np.int32)
            rel_p = np.full(t * P, -1.0, np.float32)
            src_p[:L] = s_src[e_lo:e_hi][o2]
            rel_p[:L] = (s_dst[e_lo:e_hi][o2] - nb_lo).astype(np.float32)
            off = int(moffs[g])
            meta[c_, :, off : off + t] = src_p.reshape(t, P).T
            meta[c_, :, off + t : off + 2 * t] = rel_p.reshape(t, P).T.view(np.int32)
            # block's 128 global node ids (clamped to valid range for pads)
            bn = np.minimum(nb_lo + np.arange(P), n - 1).astype(np.int32)
            meta[c_, :, off + 2 * t] = bn
            omap.append((nb_lo, n_nodes))
        out_map.append(omap)
    return cfg, meta, out_map


def build_program(cfg: Cfg, skip_p2: bool = False):
    n = cfg.n
    nc = bacc.Bacc()

    xT = nc.dram_tensor("xT", [P, n], FP32, kind="ExternalInput")
    w_of = nc.dram_tensor("w_of", [P, P], FP32, kind="ExternalInput")   # W [o,f]
    wt_fo = nc.dram_tensor("wt_fo", [P, P], FP32, kind="ExternalInput")  # W.T [f,o]
    apat = nc.dram_tensor("apat", [P, 8], FP32, kind="ExternalInput")   # [o, 8]
    iota_in = nc.dram_tensor("iota_in", [1, P], FP32, kind="ExternalInput")
    ident_in = nc.dram_tensor("ident_in", [P, P], FP32, kind="ExternalInput")
    gamma_c = nc.dram_tensor("gamma_c", [P, 1], FP32, kind="ExternalInput")
    beta_c = nc.dram_tensor("beta_c", [P, 1], FP32, kind="ExternalInput")
    meta = nc.dram_tensor(
        "meta", [P, 2 * cfg.TT + cfg.npos], I32, kind="ExternalInput")
    out = nc.dram_tensor("out", [cfg.npos * P, P], FP32, kind="ExternalOutput")

    g2 = nc.dram_tensor("g2", [n, GROW], FP32)
    ccin = nc.dram_tensor("ccin", [P, 2], FP32)
    ccout = nc.dram_tensor("ccout", [P, 2], FP32)
    scsh = nc.dram_tensor("scsh", [2, P], FP32)

    with tile.TileContext(nc) as tc, ExitStack() as ctx:
        consts = ctx.enter_context(tc.tile_pool(name="consts", bufs=1))
        p1x = ctx.enter_context(tc.tile_pool(name="p1x", bufs=3))
        p1g = ctx.enter_context(tc.tile_pool(name="p1g", bufs=3))
        p1ps = ctx.enter_context(tc.tile_pool(name="p1ps", bufs=2, space="PSUM"))
        mpool = ctx.enter_context(tc.tile_pool(name="mpool", bufs=3))
        vpool = ctx.enter_context(tc.tile_pool(name="vpool", bufs=3))
        spool = ctx.enter_context(tc.tile_pool(name="spool", bufs=3))
        adpool = ctx.enter_context(tc.tile_pool(name="adpool", bufs=3))
        stbpool = ctx.enter_context(tc.tile_pool(name="stbpool", bufs=3))
        scpool = ctx.enter_context(tc.tile_pool(name="scpool", bufs=3))
        blkps = ctx.enter_context(tc.tile_pool(name="blkps", bufs=2, space="PSUM"))
        trps = ctx.enter_context(tc.tile_pool(name="trps", bufs=1, space="PSUM"))
        adps = ctx.enter_context(tc.tile_pool(name="adps", bufs=1, space="PSUM"))
        epi = ctx.enter_context(tc.tile_pool(name="epi", bufs=4))
        opre = ctx.enter_context(tc.tile_pool(name="opre", bufs=1))
        ph3 = ctx.enter_context(tc.tile_pool(name="ph3", bufs=3))

        # ---- constants ----
        iota_sb = consts.tile([P, P], FP32)
        nc.sync.dma_start(
            out=iota_sb[:],
            in_=bass.AP(tensor=iota_in.ap().tensor, offset=0, ap=[[0, P], [1, P]]),
        )
        ident_sb = consts.tile([P, P], FP32)
        nc.sync.dma_start(out=ident_sb[:], in_=ident_in[:, :])
        rhs_sb = consts.tile([P, GROW], FP32)  # [W.T | WA_src | WA_dst]
        nc.sync.dma_start(out=rhs_sb[:, 0:P], in_=wt_fo[:, :])
        w_sb = consts.tile([P, P], FP32)
        nc.sync.dma_start(out=w_sb[:], in_=w_of[:, :])
        apat_sb = consts.tile([P, 8], FP32)
        nc.sync.dma_start(out=apat_sb[:], in_=apat[:, :])
        ones_col = consts.tile([P, 1], FP32)
        nc.vector.memset(ones_col[:], 1.0)
        gam_sb = consts.tile([P, 1], FP32)
        nc.sync.dma_start(out=gam_sb[:], in_=gamma_c[:, :])
        bet_sb = consts.tile([P, 1], FP32)
        nc.sync.dma_start(out=bet_sb[:], in_=beta_c[:, :])

        wa_ps = trps.tile([P, 8], FP32, tag="wa")
        nc.tensor.matmul(out=wa_ps[:], lhsT=w_sb[:], rhs=apat_sb[:], start=True, stop=True)
        nc.scalar.copy(out=rhs_sb[:, P : P + 8], in_=wa_ps[:])

        # ---- phase 1: G2[n] = [x W.T | a_src | a_dst] for all n ----
        # GRPL tiles per load/store DMA; matmul+copy in sub-groups of GRP
        # (PSUM bank limit), copies alternating DVE/ACT.
        GRP = 3
        GRPL = 12
        ntiles = cfg.nxt
        copy_i = 0
        tl = 0
        while tl < ntiles:
            lts = min(GRPL, ntiles - tl)
            nb = tl * P
            ncols = min(lts * P, n - nb)
            xt_sb = p1x.tile([P, GRPL * P], FP32, tag="xt")
            nc.scalar.dma_start(out=xt_sb[:, 0:ncols], in_=xT[:, nb : nb + ncols])
            g_sb = p1g.tile([P, GRPL * GROW], FP32, tag="g")
            sl = 0
            while sl < lts:
                gts = min(GRP, lts - sl)
                ps = p1ps.tile([P, GRP * GROW], FP32, tag="p1")
                for t in range(gts):
                    m = min(P, n - nb - (sl + t) * P)
                    nc.tensor.matmul(
                        out=ps[0:m, t * GROW : (t + 1) * GROW],
                        lhsT=xt_sb[:, (sl + t) * P : (sl + t) * P + m],
                        rhs=rhs_sb[:],
                        start=True,
                        stop=True,
                    )
                dst_sl = g_sb[:, sl * GROW : (sl + gts) * GROW]
                if copy_i % 2 == 0:
                    nc.scalar.copy(out=dst_sl, in_=ps[:, 0 : gts * GROW])
                else:
                    nc.vector.tensor_copy(dst_sl, ps[:, 0 : gts * GROW])
                copy_i += 1
                sl += gts
            if ncols == lts * P:
                nc.sync.dma_start(
                    out=g2[nb : nb + lts * P, :].rearrange("(t p) c -> p t c", t=lts),
                    in_=g_sb[:].rearrange("p (t c) -> p t c", c=GROW)[:, 0:lts, :],
                )
            else:
                for t in range(lts):
                    m = min(P, n - nb - t * P)
                    nc.sync.dma_start(
                        out=g2[nb + t * P : nb + t * P + m, :],
                        in_=g_sb[0:m, t * GROW : (t + 1) * GROW],
                    )
            tl += lts

        # ---- phase 2: per position ----
        stacc_sb = consts.tile([P, 2], FP32)
        opre_buf = opre.tile([P, cfg.npos * P], FP32)
        if skip_p2:
            nc.vector.memset(stacc_sb[:], 1.0)
            nc.vector.memset(opre_buf[:], 0.5)
        tmax = int(max(cfg.t_list))
        for g in range(0 if skip_p2 else cfg.npos):
            t = int(cfg.t_list[g])
            off = int(cfg.moffs[g])
            mw = 2 * t + 1
            m_sb = mpool.tile([P, 2 * tmax + 1], I32, tag="meta")
            nc.scalar.dma_start(out=m_sb[:, 0:mw], in_=meta[:, off : off + mw])
            srcidx = m_sb[:, 0:t]
            rel = m_sb[:, t : 2 * t].bitcast(FP32)
            blknode = m_sb[:, 2 * t : 2 * t + 1]

            # per-edge source rows [xt | a_src | a_dst(unused)]
            v = vpool.tile([P, tmax * GROW], FP32, tag="v")
            v3 = v[:, 0 : t * GROW].rearrange("p (t c) -> p t c", c=GROW)
            for j in range(t):
                nc.gpsimd.indirect_dma_start(
                    out=v3[:, j, :],
                    out_offset=None,
                    in_=g2[:, :],
                    in_offset=bass.IndirectOffsetOnAxis(ap=srcidx[:, j : j + 1], axis=0),
                )
            # block's own a_dst values [128, H]
            adb = adpool.tile([P, H], FP32, tag="adb")
            nc.gpsimd.indirect_dma_start(
                out=adb[:],
                out_offset=None,
                in_=g2[:, :],
                in_offset=bass.IndirectOffsetOnAxis(ap=blknode, axis=0),
                element_offset=P + H,
            )

            # S one-hot [e, n] (built first; also yields ST for a_dst lookup)
            s_t = spool.tile([P, tmax * P], FP32, tag="s")
            s3 = s_t[:, 0 : t * P].rearrange("p (t x) -> p t x", x=P)
            nc.vector.tensor_tensor(
                out=s3,
                in0=iota_sb[:].unsqueeze(1).broadcast_to((P, t, P)),
                in1=rel.unsqueeze(2).broadcast_to((P, t, P)),
                op=mybir.AluOpType.is_equal,
            )

            # per-edge a_dst via ST @ adb: ad_ps[:, 4j:4j+4] = S_j^T^T ...
            ad_ps = adps.tile([P, tmax * H], FP32, tag="adp")
            for j in range(t):
                trp = trps.tile([P, P], FP32, tag="tr")
                nc.tensor.transpose(
                    out=trp[:], in_=s3[:, j, :], identity=ident_sb[:])
                stb = stbpool.tile([P, P], FP32, tag="stb")
                nc.scalar.copy(out=stb[:], in_=trp[:])
                nc.tensor.matmul(
                    out=ad_ps[:, j * H : (j + 1) * H],
                    lhsT=stb[:], rhs=adb[:], start=True, stop=True,
                )

            # scores: s = a_src[src] + a_dst[dst]; w = exp(max(s, 0.2 s))
            sc = scpool.tile([P, tmax * H], FP32, tag="sc")
            sc3 = sc[:, 0 : t * H].rearrange("p (t h) -> p t h", h=H)
            nc.vector.tensor_tensor(
                out=sc3, in0=v3[:, :, P : P + H],
                in1=ad_ps[:, 0 : t * H].rearrange("p (t h) -> p t h", h=H),
                op=mybir.AluOpType.add,
            )
            sc2 = scpool.tile([P, tmax * H], FP32, tag="sc2")
            sc23 = sc2[:, 0 : t * H].rearrange("p (t h) -> p t h", h=H)
            nc.vector.tensor_scalar_mul(sc23, sc3, NEG)
            nc.vector.tensor_tensor(
                out=sc23, in0=sc23, in1=sc3, op=mybir.AluOpType.max
            )
            # exp -> w, written into the a_src slots of v (rhs cols 128:132)
            nc.scalar.activation(
                out=v3[:, :, P : P + H], in_=sc23,
                func=mybir.ActivationFunctionType.Exp,
            )

            # V' = w * xt (in place, per head)
            v4 = v3[:, :, 0:P].rearrange("p t (h c) -> p t h c", c=C)
            nc.vector.tensor_tensor(
                out=v4,
                in0=v4,
                in1=v3[:, :, P : P + H].unsqueeze(3).broadcast_to((P, t, H, C)),
                op=mybir.AluOpType.mult,
            )

            bps = blkps.tile([P, P + H], FP32, tag="blk")
            for j in range(t):
                nc.tensor.matmul(
                    out=bps[:],
                    lhsT=s3[:, j, :],
                    rhs=v3[:, j, 0 : P + H],
                    start=(j == 0),
                    stop=(j == t - 1),
                )

            # epilogue: out_pre = num / max(denom, tiny)
            dmax = epi.tile([P, H], FP32, tag="dmax")
            nc.vector.tensor_scalar_max(dmax[:], bps[:, P : P + H], 1e-30)
            rec = epi.tile([P, H], FP32, tag="rec")
            nc.vector.reciprocal(rec[:], dmax[:])
            op_sl = opre_buf[:, g * P : (g + 1) * P]
            nc.vector.tensor_tensor(
                out=op_sl.rearrange("p (h c) -> p h c", c=C),
                in0=bps[:, 0:P].rearrange("p (h c) -> p h c", c=C),
                in1=rec[:].unsqueeze(2).broadcast_to((P, H, C)),
                op=mybir.AluOpType.mult,
            )
            sq = epi.tile([P, P], FP32, tag="sq")
            nc.scalar.activation(
                out=sq[:], in_=op_sl, func=mybir.ActivationFunctionType.Square
            )
            stp = adps.tile([P, 2], FP32, tag="stp")
            nc.tensor.matmul(
                out=stp[:, 0:1], lhsT=op_sl, rhs=ones_col[:], start=True, stop=True,
            )
            nc.tensor.matmul(
                out=stp[:, 1:2], lhsT=sq[:], rhs=ones_col[:], start=True, stop=True,
            )
            if g == 0:
                nc.vector.tensor_copy(stacc_sb[:], stp[:])
            else:
                nc.vector.tensor_tensor(
                    out=stacc_sb[:], in0=stacc_sb[:], in1=stp[:],
                    op=mybir.AluOpType.add,
                )

        # ---- phase 3: BN stats allreduce + normalize + leaky ----
        nc.sync.dma_start(out=ccin[:, :], in_=stacc_sb[:])
        nc.gpsimd.collective_compute(
            "AllReduce",
            mybir.AluOpType.add,
            replica_groups=[list(range(cfg.ncores))],
            ins=[ccin.ap().opt()],
            outs=[ccout.ap().opt()],
        )
        gst = ph3.tile([P, 2], FP32, tag="gst")
        nc.sync.dma_start(out=gst[:], in_=ccout[:, :])

        ntot = float(cfg.shard * cfg.ncores)
        mean_t = ph3.tile([P, 1], FP32, tag="mean")
        nc.vector.tensor_scalar_mul(mean_t[:], gst[:, 0:1], 1.0 / ntot)
        m2_t = ph3.tile([P, 1], FP32, tag="m2")
        nc.vector.tensor_scalar_mul(m2_t[:], gst[:, 1:2], 1.0 / ntot)
        var_t = ph3.tile([P, 1], FP32, tag="var")
        nc.vector.tensor_tensor(out=var_t[:], in0=mean_t[:], in1=mean_t[:], op=mybir.AluOpType.mult)
        nc.vector.tensor_sub(var_t[:], m2_t[:], var_t[:])
        nc.vector.tensor_scalar_add(var_t[:], var_t[:], EPS)
        sd_t = ph3.tile([P, 1], FP32, tag="sd")
        nc.scalar.activation(out=sd_t[:], in_=var_t[:], func=mybir.ActivationFunctionType.Sqrt)
        rinv_t = ph3.tile([P, 1], FP32, tag="rinv")
        nc.vector.reciprocal(rinv_t[:], sd_t[:])
        sc_t = ph3.tile([P, 1], FP32, tag="sct")
        nc.vector.tensor_tensor(out=sc_t[:], in0=rinv_t[:], in1=gam_sb[:], op=mybir.AluOpType.mult)
        sh_t = ph3.tile([P, 1], FP32, tag="sht")
        nc.vector.tensor_tensor(out=sh_t[:], in0=mean_t[:], in1=sc_t[:], op=mybir.AluOpType.mult)
        nc.vector.tensor_sub(sh_t[:], bet_sb[:], sh_t[:])

        nc.sync.dma_start(out=scsh[0:1, :], in_=sc_t[:])
        nc.sync.dma_start(out=scsh[1:2, :], in_=sh_t[:])
        screp = consts.tile([P, P], FP32)
        nc.sync.dma_start(
            out=screp[:],
            in_=bass.AP(tensor=scsh.ap().tensor, offset=0, ap=[[0, P], [1, P]]),
        )
        shrep = consts.tile([P, P], FP32)
        nc.sync.dma_start(
            out=shrep[:],
            in_=bass.AP(tensor=scsh.ap().tensor, offset=P, ap=[[0, P], [1, P]]),
        )

        # normalize + leaky in chunks of CH positions; split ops DVE/ACT
        CH = 7
        g = 0
        while g < cfg.npos:
            k = min(CH, cfg.npos - g)
            op_sl = opre_buf[:, g * P : (g + k) * P].rearrange(
                "p (k c) -> p k c", c=P)
            t0 = ph3.tile([P, CH * P], FP32, tag="t0")
            t03 = t0[:, 0 : k * P].rearrange("p (k c) -> p k c", c=P)
            nc.vector.tensor_tensor(
                out=t03, in0=op_sl,
                in1=screp[:].unsqueeze(1).broadcast_to((P, k, P)),
                op=mybir.AluOpType.mult)
            nc.vector.tensor_tensor(
                out=t03, in0=t03,
                in1=shrep[:].unsqueeze(1).broadcast_to((P, k, P)),
                op=mybir.AluOpType.add)
            t1 = ph3.tile([P, CH * P], FP32, tag="t1")
            nc.vector.tensor_scalar_mul(t1[:, 0 : k * P], t0[:, 0 : k * P], NEG)
            nc.vector.tensor_tensor(
                out=t1[:, 0 : k * P], in0=t1[:, 0 : k * P],
                in1=t0[:, 0 : k * P], op=mybir.AluOpType.max)
            nc.sync.dma_start(
                out=out[g * P : (g + k) * P, :].rearrange("(k p) c -> p k c", k=k),
                in_=t1[:, 0 : k * P].rearrange("p (k c) -> p k c", c=P),
            )
            g += k

    nc.compile()
    return nc


def make_inputs(x, W, att_src, att_dst, gamma, beta, meta, cfg: Cfg):
    x = np.asarray(x, np.float32)
    W = np.asarray(W, np.float32)
    att_src = np.asarray(att_src, np.float32)
    att_dst = np.asarray(att_dst, np.float32)
    apat = np.zeros((P, 8), np.float32)
    for h in range(H):
        apat[h * C : (h + 1) * C, h] = att_src[h]
        apat[h * C : (h + 1) * C, 4 + h] = att_dst[h]
    xT = np.ascontiguousarray(x.T)
    wt = np.ascontiguousarray(W.T)
    iota = np.arange(P, dtype=np.float32).reshape(1, P)
    ident = np.eye(P, dtype=np.float32)
    gam = np.asarray(gamma, np.float32).reshape(P, 1)
    bet = np.asarray(beta, np.float32).reshape(P, 1)
    in_maps = []
    for c_ in range(cfg.ncores):
        in_maps.append(
            {
                "xT": xT,
                "w_of": W,
                "wt_fo": wt,
                "apat": apat,
                "iota_in": iota,
                "ident_in": ident,
                "gamma_c": gam,
                "beta_c": bet,
                "meta": np.ascontiguousarray(meta[c_]),
            }
        )
    return in_maps


def assemble_output(core_outs, out_map, cfg: Cfg, n):
    full = np.empty((n, P), np.float32)
    for c_ in range(cfg.ncores):
        for g, (nb_lo, n_valid) in enumerate(out_map[c_]):
            if n_valid > 0:
                full[nb_lo : nb_lo + n_valid] = core_outs[c_][g * P : g * P + n_valid]
    return full


def kernel(**inputs) -> np.ndarray:
    from concourse.bass_utils import run_bass_kernel_spmd

    cfg, meta, out_map = preprocess(inputs["edge_index"])
    nc = build_program(cfg)
    in_maps = make_inputs(
        inputs["x"], inputs["W"], inputs["att_src"], inputs["att_dst"],
        inputs["gamma"], inputs["beta"], meta, cfg,
    )
    res = run_bass_kernel_spmd(nc, in_maps, core_ids=list(range(NCORES)))
    core_outs = [res.results[c_]["out"] for c_ in range(NCORES)]
    return assemble_output(core_outs, out_map, cfg, N)

